# revision 103
# baseline (speedup 1.0000x reference)
"""LookupConv2d Trainium2 kernel.

Math: out = conv2d(x, W), W[o] = sum_s coeff[o,s] * dictionary[idx[o,s]].
Factorization: W = M @ D where M[o,d] = sum_{s: idx[o,s]=d} coeff[o,s] is a
(512, 100) scatter of the coefficients.  Then
    out = M @ conv2d(x, dictionary)
i.e. a 100-channel conv (23 GFLOP) followed by a 1x1 512x100 mix (5 GFLOP)
instead of a 512-channel conv (118 GFLOP) -- 4.2x fewer FLOPs.

Precision: single-pass bf16 throughout (1 row/cycle on the PE), fp32 PSUM
accumulation.  Measured end-to-end rel err vs the fp32 reference is ~3.6e-3,
comfortably inside the 2e-2 gate; the output is stored bf16 and widened to
fp32 on the host.

Orientation: x is stored padded and FLATTENED (58x58 per image/cin-block) so
any 128 consecutive output pixels form a contiguous stationary patch for
every conv tap.  Per 128-pixel chunk the PE runs 18 accumulating matmuls
with the x patch stationary and the 100 dictionary taps moving (1800 rows vs
2304 for the dict-stationary orientation: the PE's full 128x128 array is
active), one transpose (y^T -> y via the identity trick), and ONE mix matmul
streaming all 512 output channels.  That is 2440 PE rows per 128 pixels vs
2816 the other way.  Output leaves in [pixel, channel] layout; the host
strips the pad pixels and transposes -- layout only, no arithmetic.

Schedule: 3-stage software pipeline on the PE (conv(c) / transpose(c-1) /
mix(c-2)) with a p-state warmup so the Tensor engine reaches full clock
(2.4GHz) before real work.  Weights+m ride the ACT HWDGE queue, x chunks and
output DMAs ride SP.

Sharding: data-parallel over batch N=16 -> 2 images per core on 8 cores.
The dictionary tap matrices and M^T are small and replicated.
"""

import numpy as np

N_CORES = 8
IMGS_PER_CORE = 2
CIN = 256
COUT = 512
NDICT = 100
H = W = 56
HP = WP = 58  # padded
FLAT = HP * WP  # 3364 padded pixels per (img, cin-block)
FLAT_X = 3456   # x padded further so all 26 uniform chunks stay in bounds
NCHUNK = 26     # 128-pixel chunks per image (last is partially valid)
S = 3  # lookup sparsity

# input flat-range chunks (start, len) per image
CHUNKSF0 = [(0, 384), (384, 560), (944, 1120), (2064, 1120), (3184, 272)]
CHUNKSF1 = [(0, 1200), (1200, 1200), (2400, 1056)]

TRACE = False  # set by test.py to get a profile
_LAST_RESULTS = {}  # test.py reads exec_time_ns from here


def _build_program():
    import concourse.bacc as bacc
    import concourse.mybir as mybir
    import concourse.tile as tile
    from concourse import masks

    f32 = mybir.dt.float32
    bf16 = mybir.dt.bfloat16

    nc = bacc.Bacc("TRN2", target_bir_lowering=False, debug=False)

    # host pre-arranges x into [cin_in_block, img, cblk, padded-flat pixel]
    x_d = nc.dram_tensor("x", (128, IMGS_PER_CORE, 2, FLAT_X), bf16,
                         kind="ExternalInput")
    w_d = nc.dram_tensor("w", (128, 2 * 9 * NDICT), bf16, kind="ExternalInput")
    m_d = nc.dram_tensor("m", (NDICT, COUT), bf16, kind="ExternalInput")
    # [img, chunk, pixel-in-chunk, cout]; host strips pads and transposes
    out_d = nc.dram_tensor("out", (IMGS_PER_CORE, NCHUNK, 128, COUT), bf16,
                           kind="ExternalOutput")

    with tile.TileContext(nc) as tc:
        with (
            tc.tile_pool(name="consts", bufs=1) as consts,
            tc.tile_pool(name="xpool", bufs=1) as xpool,
            tc.tile_pool(name="ytpool", bufs=12) as ytpool,
            tc.tile_pool(name="ypool", bufs=12) as ypool,
            tc.tile_pool(name="opool", bufs=10) as opool,
            tc.tile_pool(name="psum_yt", bufs=3, space="PSUM") as pyt_pool,
            tc.tile_pool(name="psum_tr", bufs=2, space="PSUM") as ptr_pool,
            tc.tile_pool(name="psum_ot", bufs=3, space="PSUM") as pot_pool,
        ):
            w_sb = consts.tile([128, 2 * 9 * NDICT], bf16)
            x_sb = xpool.tile([128, IMGS_PER_CORE, 2, FLAT_X], bf16,
                              tag="x_sb")
            m_sb = consts.tile([NDICT, COUT], bf16)
            # the first conv chunk consumes all 18 taps within ~750ns, so the
            # whole weight tensor is startup-critical: w leads the SP queue
            # (earliest transfer slots), x rides ACT
            nc.sync.dma_start(w_sb[:, :9 * NDICT], w_d[:, :9 * NDICT])
            nc.scalar.dma_start(x_sb[:, 0, :, 0:384], x_d[:, 0, :, 0:384])
            nc.sync.dma_start(w_sb[:, 9 * NDICT:], w_d[:, 9 * NDICT:])
            nc.scalar.dma_start(x_sb[:, 0, :, 384:944], x_d[:, 0, :, 384:944])
            nc.scalar.dma_start(m_sb[:], m_d[:])

            ident = consts.tile([128, 128], bf16)
            masks.make_identity(nc, ident[:])

            # PE p-state warmup: the Tensor engine clocks 1.2GHz until it has
            # been busy ~3us.  Burn the startup DMA window with tiny matmuls
            # on a zeroed scratch tile so real work starts at full clock.
            scratch = consts.tile([128, 64], bf16)
            nc.vector.memset(scratch[:], 0.0)
            wpsum = pyt_pool.tile([128, NDICT], f32, tag="yt")
            for _ in range(64):
                nc.tensor.matmul(wpsum[:64, :64], scratch[:], scratch[:],
                                 start=True, stop=True)

            for img, chunks in ((0, CHUNKSF0), (1, CHUNKSF1)):
                for a, ln in chunks:
                    if img == 0 and a < 944:
                        continue  # already issued above
                    nc.scalar.dma_start(x_sb[:, img, :, a:a + ln],
                                        x_d[:, img, :, a:a + ln])

            def emit_conv(img, s0):
                pyt = pyt_pool.tile([128, NDICT], f32, tag="yt")
                for k in range(18):
                    cb, t9 = divmod(k, 9)
                    ti, tj = divmod(t9, 3)
                    off = s0 + ti * WP + tj
                    nc.tensor.matmul(
                        pyt[:], x_sb[:, img, cb, off:off + 128],
                        w_sb[:, k * NDICT:(k + 1) * NDICT],
                        start=(k == 0), stop=(k == 17))
                yt = ytpool.tile([128, NDICT], bf16, tag="yt_sb")
                nc.vector.tensor_copy(yt[:], pyt[:])
                return yt

            def emit_tr(yt):
                ptr = ptr_pool.tile([NDICT, 128], bf16, tag="tr")
                nc.tensor.transpose(ptr[:], yt[:, :NDICT], ident[:])
                y = ypool.tile([NDICT, 128], bf16, tag="y_sb")
                nc.scalar.copy(y[:], ptr[:])
                return y

            # output DMAs cover chunk PAIRS (halves the HWDGE launch load)
            # via a [i, p, c, o] view whose iteration order matches SBUF;
            # each image's last pair goes as two singles to keep the drain
            # chain short
            out_v = out_d.rearrange("i c p o -> i p c o")
            opair = [None]

            def emit_mix(y, img, ci):
                pot = pot_pool.tile([128, COUT], f32, tag="ot")
                nc.tensor.matmul(pot[:], y[:], m_sb[:], start=True, stop=True)
                if ci % 2 == 0:
                    o_new = opool.tile([128, 2, COUT], bf16, tag="o")
                    opair[0] = o_new
                o = opair[0]
                sp = 320 if (img == 1 and ci >= NCHUNK - 3) else 160
                nc.vector.tensor_copy(o[:, ci % 2, :sp], pot[:, :sp])
                nc.scalar.copy(o[:, ci % 2, sp:], pot[:, sp:])
                if ci >= NCHUNK - 2:
                    eng = nc.scalar if (ci == NCHUNK - 1 and img == 1) \
                        else nc.sync
                    np_rows = 46 if ci == NCHUNK - 1 else 128
                    eng.dma_start(out_v[img, :np_rows, ci:ci + 1, :],
                                  o[:np_rows, ci % 2:ci % 2 + 1, :])
                elif ci % 2 == 1:
                    nc.sync.dma_start(out_v[img, :, ci - 1:ci + 1, :], o[:])

            items = [(img, 128 * c) for img in range(IMGS_PER_CORE)
                     for c in range(NCHUNK)]
            # 3-stage pipeline: conv(c) | transpose(c-1) | mix(c-2)
            st_tr = []   # awaiting transpose: (yt, img, s0)
            st_mix = []  # awaiting mix: (y, img, s0)
            for i, (img, s0) in enumerate(items):
                yt = emit_conv(img, s0)
                st_tr.append((yt, img, s0))
                if len(st_tr) > 1:
                    yt1, im1, sp1 = st_tr.pop(0)
                    st_mix.append((emit_tr(yt1), im1, sp1))
                if len(st_mix) > 1:
                    y1, im1, sp1 = st_mix.pop(0)
                    emit_mix(y1, im1, sp1 // 128)
            yt1, im1, sp1 = st_tr.pop(0)
            st_mix.append((emit_tr(yt1), im1, sp1))
            for y1, im1, sp1 in st_mix:
                emit_mix(y1, im1, sp1 // 128)

    nc.compile()
    return nc


_NC_CACHE = None


def kernel(x, dictionary, lookup_indices, lookup_coefficients):
    global _NC_CACHE
    import ml_dtypes
    from concourse import bass_utils

    bf16 = ml_dtypes.bfloat16

    x = np.asarray(x, dtype=np.float32)
    dictionary = np.asarray(dictionary, dtype=np.float32)
    idx = np.asarray(lookup_indices).astype(np.int64)
    coef = np.asarray(lookup_coefficients, dtype=np.float32)

    # M^T[d, o] = sum_s coeff[o, s] * [idx[o, s] == d]
    mt = np.zeros((NDICT, COUT), np.float32)
    np.add.at(mt, (idx.reshape(-1),
                   np.repeat(np.arange(COUT), S)), coef.reshape(-1))

    # wt[c_in_block, (cblk, ti, tj, d)] = dictionary[d, cblk*128+c, ti, tj]
    wt = np.ascontiguousarray(
        dictionary.reshape(NDICT, 2, 128, 3, 3).transpose(2, 1, 3, 4, 0)
    ).reshape(128, 2 * 9 * NDICT)

    # x -> [core, cin_in_block, img, cblk, padded-flat] (the SBUF layout, so
    # the device DMA is a plain contiguous copy)
    xp = np.pad(x, ((0, 0), (0, 0), (1, 1), (1, 1)))
    xp = xp.reshape(N_CORES, IMGS_PER_CORE, 2, 128, FLAT)
    xp = np.pad(xp, ((0, 0),) * 4 + ((0, FLAT_X - FLAT),))
    xp = np.ascontiguousarray(xp.transpose(0, 3, 1, 2, 4))

    if _NC_CACHE is None:
        _NC_CACHE = _build_program()
    nc = _NC_CACHE

    in_maps = [{"x": xp[i].astype(bf16), "w": wt.astype(bf16),
                "m": mt.astype(bf16)} for i in range(N_CORES)]
    try:
        res = bass_utils.run_bass_kernel_spmd(
            nc, in_maps, core_ids=list(range(N_CORES)), trace=TRACE)
    except ModuleNotFoundError:
        # no axon NTFF profile hook in this environment
        res = bass_utils.run_bass_kernel_spmd(
            nc, in_maps, core_ids=list(range(N_CORES)), trace=False)
    _LAST_RESULTS["res"] = res

    # [core, img, chunk, 128, cout] -> strip pads, (16, 512, 56, 56) fp32
    out = np.stack([r["out"] for r in res.results], axis=0)
    out = out.reshape(N_CORES, IMGS_PER_CORE, NCHUNK * 128, COUT)
    valid = (np.arange(H)[:, None] * WP + np.arange(W)[None, :]).ravel()
    out = out.astype(np.float32)[:, :, valid, :]
    return np.ascontiguousarray(
        out.transpose(0, 1, 3, 2)).reshape(16, COUT, H, W)


# revision 104
# speedup vs baseline: 1.0175x; 1.0175x over previous
"""LookupConv2d Trainium2 kernel.

Math: out = conv2d(x, W), W[o] = sum_s coeff[o,s] * dictionary[idx[o,s]].
Factorization: W = M @ D where M[o,d] = sum_{s: idx[o,s]=d} coeff[o,s] is a
(512, 100) scatter of the coefficients.  Then
    out = M @ conv2d(x, dictionary)
i.e. a 100-channel conv (23 GFLOP) followed by a 1x1 512x100 mix (5 GFLOP)
instead of a 512-channel conv (118 GFLOP) -- 4.2x fewer FLOPs.

Precision: single-pass bf16 throughout (1 row/cycle on the PE), fp32 PSUM
accumulation.  Measured end-to-end rel err vs the fp32 reference is ~3.6e-3,
comfortably inside the 2e-2 gate; the output is stored bf16 and widened to
fp32 on the host.

Orientation: x is stored padded and FLATTENED (58x58 per image/cin-block) so
any 128 consecutive output pixels form a contiguous stationary patch for
every conv tap.  Per 128-pixel chunk the PE runs 18 accumulating matmuls
with the x patch stationary and the 100 dictionary taps moving (1800 rows vs
2304 for the dict-stationary orientation: the PE's full 128x128 array is
active), one transpose (y^T -> y via the identity trick), and ONE mix matmul
streaming all 512 output channels.  That is 2440 PE rows per 128 pixels vs
2816 the other way.  Output leaves in [pixel, channel] layout; the host
strips the pad pixels and transposes -- layout only, no arithmetic.

Schedule: 3-stage software pipeline on the PE (conv(c) / transpose(c-1) /
mix(c-2)) with a p-state warmup so the Tensor engine reaches full clock
(2.4GHz) before real work.  Weights+m ride the ACT HWDGE queue, x chunks and
output DMAs ride SP.

Sharding: data-parallel over batch N=16 -> 2 images per core on 8 cores.
The dictionary tap matrices and M^T are small and replicated.
"""

import numpy as np

N_CORES = 8
IMGS_PER_CORE = 2
CIN = 256
COUT = 512
NDICT = 100
H = W = 56
HP = WP = 58  # padded (reference geometry)
# x rides a 57-stride layout: adjacent rows SHARE one zero pad column (the
# right pad of row r is the left pad of row r+1), so the output span is
# 57*55+56=3191 pixels -> 25 chunks instead of 26 (saves 2x2440 PE rows)
STR = 57
FLAT_X = 3328   # 58*57=3306 rounded up so all 25 uniform chunks stay in bounds
NCHUNK = 25     # 128-pixel chunks per image (last is partially valid)
S = 3  # lookup sparsity

# input flat-range chunks (start, len) per image
CHUNKSF0 = [(0, 384), (384, 560), (944, 1120), (2064, 1120), (3184, 144)]
CHUNKSF1 = [(0, 1200), (1200, 1200), (2400, 928)]

TRACE = False  # set by test.py to get a profile
_LAST_RESULTS = {}  # test.py reads exec_time_ns from here


def _build_program():
    import concourse.bacc as bacc
    import concourse.mybir as mybir
    import concourse.tile as tile
    from concourse import masks

    f32 = mybir.dt.float32
    bf16 = mybir.dt.bfloat16

    nc = bacc.Bacc("TRN2", target_bir_lowering=False, debug=False)

    # host pre-arranges x into [cin_in_block, img, cblk, padded-flat pixel]
    x_d = nc.dram_tensor("x", (128, IMGS_PER_CORE, 2, FLAT_X), bf16,
                         kind="ExternalInput")
    w_d = nc.dram_tensor("w", (128, 2 * 9 * NDICT), bf16, kind="ExternalInput")
    m_d = nc.dram_tensor("m", (NDICT, COUT), bf16, kind="ExternalInput")
    # [img, chunk, pixel-in-chunk, cout]; host strips pads and transposes
    out_d = nc.dram_tensor("out", (IMGS_PER_CORE, NCHUNK, 128, COUT), bf16,
                           kind="ExternalOutput")

    with tile.TileContext(nc) as tc:
        with (
            tc.tile_pool(name="consts", bufs=1) as consts,
            tc.tile_pool(name="xpool", bufs=1) as xpool,
            tc.tile_pool(name="ytpool", bufs=12) as ytpool,
            tc.tile_pool(name="ypool", bufs=12) as ypool,
            tc.tile_pool(name="opool", bufs=10) as opool,
            tc.tile_pool(name="psum_yt", bufs=3, space="PSUM") as pyt_pool,
            tc.tile_pool(name="psum_tr", bufs=2, space="PSUM") as ptr_pool,
            tc.tile_pool(name="psum_ot", bufs=3, space="PSUM") as pot_pool,
        ):
            w_sb = consts.tile([128, 2 * 9 * NDICT], bf16)
            x_sb = xpool.tile([128, IMGS_PER_CORE, 2, FLAT_X], bf16,
                              tag="x_sb")
            m_sb = consts.tile([NDICT, COUT], bf16)
            # the first conv chunk consumes all 18 taps within ~750ns, so the
            # whole weight tensor is startup-critical: w leads the SP queue
            # (earliest transfer slots), x rides ACT
            nc.sync.dma_start(w_sb[:, :9 * NDICT], w_d[:, :9 * NDICT])
            nc.scalar.dma_start(x_sb[:, 0, :, 0:384], x_d[:, 0, :, 0:384])
            nc.sync.dma_start(w_sb[:, 9 * NDICT:], w_d[:, 9 * NDICT:])
            nc.scalar.dma_start(x_sb[:, 0, :, 384:944], x_d[:, 0, :, 384:944])
            nc.scalar.dma_start(m_sb[:], m_d[:])

            ident = consts.tile([128, 128], bf16)
            masks.make_identity(nc, ident[:])

            # PE p-state warmup: the Tensor engine clocks 1.2GHz until it has
            # been busy ~3us.  Burn the startup DMA window with tiny matmuls
            # on a zeroed scratch tile so real work starts at full clock.
            scratch = consts.tile([128, 64], bf16)
            nc.vector.memset(scratch[:], 0.0)
            wpsum = pyt_pool.tile([128, NDICT], f32, tag="yt")
            for _ in range(64):
                nc.tensor.matmul(wpsum[:64, :64], scratch[:], scratch[:],
                                 start=True, stop=True)

            for img, chunks in ((0, CHUNKSF0), (1, CHUNKSF1)):
                for a, ln in chunks:
                    if img == 0 and a < 944:
                        continue  # already issued above
                    nc.scalar.dma_start(x_sb[:, img, :, a:a + ln],
                                        x_d[:, img, :, a:a + ln])

            def emit_conv(img, s0):
                pyt = pyt_pool.tile([128, NDICT], f32, tag="yt")
                for k in range(18):
                    cb, t9 = divmod(k, 9)
                    ti, tj = divmod(t9, 3)
                    off = s0 + ti * STR + tj
                    nc.tensor.matmul(
                        pyt[:], x_sb[:, img, cb, off:off + 128],
                        w_sb[:, k * NDICT:(k + 1) * NDICT],
                        start=(k == 0), stop=(k == 17))
                yt = ytpool.tile([128, NDICT], bf16, tag="yt_sb")
                nc.vector.tensor_copy(yt[:], pyt[:])
                return yt

            def emit_tr(yt):
                ptr = ptr_pool.tile([NDICT, 128], bf16, tag="tr")
                nc.tensor.transpose(ptr[:], yt[:, :NDICT], ident[:])
                y = ypool.tile([NDICT, 128], bf16, tag="y_sb")
                nc.scalar.copy(y[:], ptr[:])
                return y

            # output DMAs cover chunk PAIRS (halves the HWDGE launch load)
            # via a [i, p, c, o] view whose iteration order matches SBUF;
            # each image's last pair goes as two singles to keep the drain
            # chain short
            out_v = out_d.rearrange("i c p o -> i p c o")
            opair = [None]

            def emit_mix(y, img, ci):
                pot = pot_pool.tile([128, COUT], f32, tag="ot")
                nc.tensor.matmul(pot[:], y[:], m_sb[:], start=True, stop=True)
                if ci % 2 == 0:
                    o_new = opool.tile([128, 2, COUT], bf16, tag="o")
                    opair[0] = o_new
                o = opair[0]
                sp = 320 if (img == 1 and ci >= NCHUNK - 3) else 160
                nc.vector.tensor_copy(o[:, ci % 2, :sp], pot[:, :sp])
                nc.scalar.copy(o[:, ci % 2, sp:], pot[:, sp:])
                if ci >= NCHUNK - 3:
                    eng = nc.scalar if ((ci == NCHUNK - 1 and img == 1)
                                        or ci == NCHUNK - 2) else nc.sync
                    np_rows = 119 if ci == NCHUNK - 1 else 128
                    eng.dma_start(out_v[img, :np_rows, ci:ci + 1, :],
                                  o[:np_rows, ci % 2:ci % 2 + 1, :])
                elif ci % 2 == 1:
                    nc.sync.dma_start(out_v[img, :, ci - 1:ci + 1, :], o[:])

            items = [(img, 128 * c) for img in range(IMGS_PER_CORE)
                     for c in range(NCHUNK)]
            # 3-stage pipeline: conv(c) | transpose(c-1) | mix(c-2)
            st_tr = []   # awaiting transpose: (yt, img, s0)
            st_mix = []  # awaiting mix: (y, img, s0)
            for i, (img, s0) in enumerate(items):
                yt = emit_conv(img, s0)
                st_tr.append((yt, img, s0))
                if len(st_tr) > 1:
                    yt1, im1, sp1 = st_tr.pop(0)
                    st_mix.append((emit_tr(yt1), im1, sp1))
                if len(st_mix) > 1:
                    y1, im1, sp1 = st_mix.pop(0)
                    emit_mix(y1, im1, sp1 // 128)
            yt1, im1, sp1 = st_tr.pop(0)
            st_mix.append((emit_tr(yt1), im1, sp1))
            for y1, im1, sp1 in st_mix:
                emit_mix(y1, im1, sp1 // 128)

    nc.compile()
    return nc


_NC_CACHE = None


def kernel(x, dictionary, lookup_indices, lookup_coefficients):
    global _NC_CACHE
    import ml_dtypes
    from concourse import bass_utils

    bf16 = ml_dtypes.bfloat16

    x = np.asarray(x, dtype=np.float32)
    dictionary = np.asarray(dictionary, dtype=np.float32)
    idx = np.asarray(lookup_indices).astype(np.int64)
    coef = np.asarray(lookup_coefficients, dtype=np.float32)

    # M^T[d, o] = sum_s coeff[o, s] * [idx[o, s] == d]
    mt = np.zeros((NDICT, COUT), np.float32)
    np.add.at(mt, (idx.reshape(-1),
                   np.repeat(np.arange(COUT), S)), coef.reshape(-1))

    # wt[c_in_block, (cblk, ti, tj, d)] = dictionary[d, cblk*128+c, ti, tj]
    wt = np.ascontiguousarray(
        dictionary.reshape(NDICT, 2, 128, 3, 3).transpose(2, 1, 3, 4, 0)
    ).reshape(128, 2 * 9 * NDICT)

    # x -> [core, cin_in_block, img, cblk, padded-flat] (the SBUF layout, so
    # the device DMA is a plain contiguous copy)
    xp = np.zeros((16, CIN, HP, STR), np.float32)
    xp[:, :, 1:57, 1:57] = x
    xp = xp.reshape(16, CIN, HP * STR)
    xp = np.pad(xp, ((0, 0), (0, 0), (0, FLAT_X - HP * STR)))
    xp = xp.reshape(N_CORES, IMGS_PER_CORE, 2, 128, FLAT_X)
    xp = np.ascontiguousarray(xp.transpose(0, 3, 1, 2, 4))

    if _NC_CACHE is None:
        _NC_CACHE = _build_program()
    nc = _NC_CACHE

    in_maps = [{"x": xp[i].astype(bf16), "w": wt.astype(bf16),
                "m": mt.astype(bf16)} for i in range(N_CORES)]
    try:
        res = bass_utils.run_bass_kernel_spmd(
            nc, in_maps, core_ids=list(range(N_CORES)), trace=TRACE)
    except ModuleNotFoundError:
        # no axon NTFF profile hook in this environment
        res = bass_utils.run_bass_kernel_spmd(
            nc, in_maps, core_ids=list(range(N_CORES)), trace=False)
    _LAST_RESULTS["res"] = res

    # [core, img, chunk, 128, cout] -> strip pads, (16, 512, 56, 56) fp32
    out = np.stack([r["out"] for r in res.results], axis=0)
    out = out.reshape(N_CORES, IMGS_PER_CORE, NCHUNK * 128, COUT)
    valid = (np.arange(H)[:, None] * STR + np.arange(W)[None, :]).ravel()
    out = out.astype(np.float32)[:, :, valid, :]
    return np.ascontiguousarray(
        out.transpose(0, 1, 3, 2)).reshape(16, COUT, H, W)


# revision 105
# speedup vs baseline: 1.0312x; 1.0135x over previous
"""LookupConv2d Trainium2 kernel.

Math: out = conv2d(x, W), W[o] = sum_s coeff[o,s] * dictionary[idx[o,s]].
Factorization: W = M @ D where M[o,d] = sum_{s: idx[o,s]=d} coeff[o,s] is a
(512, 100) scatter of the coefficients.  Then
    out = M @ conv2d(x, dictionary)
i.e. a 100-channel conv (23 GFLOP) followed by a 1x1 512x100 mix (5 GFLOP)
instead of a 512-channel conv (118 GFLOP) -- 4.2x fewer FLOPs.

Precision: single-pass bf16 throughout (1 row/cycle on the PE), fp32 PSUM
accumulation.  Measured end-to-end rel err vs the fp32 reference is ~3.6e-3,
comfortably inside the 2e-2 gate; the output is stored bf16 and widened to
fp32 on the host.

Orientation: x is stored padded and FLATTENED (58x58 per image/cin-block) so
any 128 consecutive output pixels form a contiguous stationary patch for
every conv tap.  Per 128-pixel chunk the PE runs 18 accumulating matmuls
with the x patch stationary and the 100 dictionary taps moving (1800 rows vs
2304 for the dict-stationary orientation: the PE's full 128x128 array is
active), one transpose (y^T -> y via the identity trick), and ONE mix matmul
streaming all 512 output channels.  That is 2440 PE rows per 128 pixels vs
2816 the other way.  Output leaves in [pixel, channel] layout; the host
strips the pad pixels and transposes -- layout only, no arithmetic.

Schedule: 3-stage software pipeline on the PE (conv(c) / transpose(c-1) /
mix(c-2)) with a p-state warmup so the Tensor engine reaches full clock
(2.4GHz) before real work.  Weights+m ride the ACT HWDGE queue, x chunks and
output DMAs ride SP.

Sharding: data-parallel over batch N=16 -> 2 images per core on 8 cores.
The dictionary tap matrices and M^T are small and replicated.
"""

import numpy as np

N_CORES = 8
IMGS_PER_CORE = 2
CIN = 256
COUT = 512
NDICT = 100
H = W = 56
HP = WP = 58  # padded (reference geometry)
# x rides a 57-stride layout: adjacent rows SHARE one zero pad column (the
# right pad of row r is the left pad of row r+1), so the output span is
# 57*55+56=3191 pixels -> 25 chunks instead of 26 (saves 2x2440 PE rows)
STR = 57
FLAT_X = 3328   # 58*57=3306 rounded up so all 25 uniform chunks stay in bounds
NCHUNK = 25     # 128-pixel chunks per image (last is partially valid)
S = 3  # lookup sparsity

# input flat-range chunks (start, len) per image
CHUNKSF0 = [(0, 384), (384, 560), (944, 1120), (2064, 1120), (3184, 144)]
CHUNKSF1 = [(0, 1200), (1200, 1200), (2400, 928)]

TRACE = False  # set by test.py to get a profile
_LAST_RESULTS = {}  # test.py reads exec_time_ns from here


def _build_program():
    import concourse.bacc as bacc
    import concourse.mybir as mybir
    import concourse.tile as tile
    from concourse import masks

    f32 = mybir.dt.float32
    bf16 = mybir.dt.bfloat16

    nc = bacc.Bacc("TRN2", target_bir_lowering=False, debug=False)

    # host pre-arranges x into [cin_in_block, img, cblk, padded-flat pixel]
    x_d = nc.dram_tensor("x", (128, IMGS_PER_CORE, 2, FLAT_X), bf16,
                         kind="ExternalInput")
    w_d = nc.dram_tensor("w", (128, 2 * 9 * NDICT), bf16, kind="ExternalInput")
    m_d = nc.dram_tensor("m", (NDICT, COUT), bf16, kind="ExternalInput")
    # [img, chunk, pixel-in-chunk, cout]; host strips pads and transposes
    out_d = nc.dram_tensor("out", (IMGS_PER_CORE, NCHUNK, 128, COUT), bf16,
                           kind="ExternalOutput")

    with tile.TileContext(nc) as tc:
        with (
            tc.tile_pool(name="consts", bufs=1) as consts,
            tc.tile_pool(name="xpool", bufs=1) as xpool,
            tc.tile_pool(name="ytpool", bufs=12) as ytpool,
            tc.tile_pool(name="ypool", bufs=12) as ypool,
            tc.tile_pool(name="opool", bufs=10) as opool,
            tc.tile_pool(name="psum_yt", bufs=3, space="PSUM") as pyt_pool,
            tc.tile_pool(name="psum_tr", bufs=2, space="PSUM") as ptr_pool,
            tc.tile_pool(name="psum_ot", bufs=3, space="PSUM") as pot_pool,
        ):
            w_sb = consts.tile([128, 2 * 9 * NDICT], bf16)
            x_sb = xpool.tile([128, IMGS_PER_CORE, 2, FLAT_X], bf16,
                              tag="x_sb")
            m_sb = consts.tile([NDICT, COUT], bf16)
            # the first conv chunk consumes all 18 taps within ~750ns, so the
            # whole weight tensor is startup-critical: w leads the SP queue
            # (earliest transfer slots), x rides ACT
            nc.sync.dma_start(w_sb[:, :9 * NDICT], w_d[:, :9 * NDICT])
            nc.scalar.dma_start(x_sb[:, 0, :, 0:384], x_d[:, 0, :, 0:384])
            nc.sync.dma_start(w_sb[:, 9 * NDICT:], w_d[:, 9 * NDICT:])
            nc.scalar.dma_start(x_sb[:, 0, :, 384:944], x_d[:, 0, :, 384:944])
            nc.scalar.dma_start(m_sb[:], m_d[:])

            ident = consts.tile([128, 128], bf16)
            masks.make_identity(nc, ident[:])

            # PE p-state warmup: the Tensor engine clocks 1.2GHz until it has
            # been busy ~3us.  Burn the startup DMA window with tiny matmuls
            # on a zeroed scratch tile so real work starts at full clock.
            scratch = consts.tile([128, 64], bf16)
            nc.vector.memset(scratch[:], 0.0)
            wpsum = pyt_pool.tile([128, NDICT], f32, tag="yt")
            for _ in range(64):
                nc.tensor.matmul(wpsum[:64, :64], scratch[:], scratch[:],
                                 start=True, stop=True)

            for img, chunks in ((0, CHUNKSF0), (1, CHUNKSF1)):
                for a, ln in chunks:
                    if img == 0 and a < 944:
                        continue  # already issued above
                    nc.scalar.dma_start(x_sb[:, img, :, a:a + ln],
                                        x_d[:, img, :, a:a + ln])

            def emit_conv(img, s0):
                pyt = pyt_pool.tile([128, NDICT], f32, tag="yt")
                for k in range(18):
                    cb, t9 = divmod(k, 9)
                    ti, tj = divmod(t9, 3)
                    off = s0 + ti * STR + tj
                    nc.tensor.matmul(
                        pyt[:], x_sb[:, img, cb, off:off + 128],
                        w_sb[:, k * NDICT:(k + 1) * NDICT],
                        start=(k == 0), stop=(k == 17))
                yt = ytpool.tile([128, NDICT], bf16, tag="yt_sb")
                nc.vector.tensor_copy(yt[:], pyt[:])
                return yt

            def emit_tr(yt):
                ptr = ptr_pool.tile([NDICT, 128], bf16, tag="tr")
                nc.tensor.transpose(ptr[:], yt[:, :NDICT], ident[:])
                y = ypool.tile([NDICT, 128], bf16, tag="y_sb")
                nc.scalar.copy(y[:], ptr[:])
                return y

            # output DMAs cover chunk PAIRS (halves the HWDGE launch load)
            # via a [i, p, c, o] view whose iteration order matches SBUF;
            # each image's last pair goes as two singles to keep the drain
            # chain short
            out_v = out_d.rearrange("i c p o -> i p c o")
            opair = [None]

            def emit_mix(y, img, ci):
                pot = pot_pool.tile([128, COUT], f32, tag="ot")
                nc.tensor.matmul(pot[:], y[:], m_sb[:], start=True, stop=True)
                if ci % 2 == 0:
                    o_new = opool.tile([128, 2, COUT], bf16, tag="o")
                    opair[0] = o_new
                o = opair[0]
                sp = 320 if (img == 1 and ci >= NCHUNK - 3) else 160
                nc.vector.tensor_copy(o[:, ci % 2, :sp], pot[:, :sp])
                nc.scalar.copy(o[:, ci % 2, sp:], pot[:, sp:])
                if ci == NCHUNK - 1:
                    eng = nc.scalar if img == 1 else nc.sync
                    eng.dma_start(out_v[img, :119, ci:ci + 1, :],
                                  o[:119, ci % 2:ci % 2 + 1, :])
                elif ci % 2 == 1:
                    nc.sync.dma_start(out_v[img, :, ci - 1:ci + 1, :], o[:])

            items = [(img, 128 * c) for img in range(IMGS_PER_CORE)
                     for c in range(NCHUNK)]
            # 3-stage pipeline: conv(c) | transpose(c-1) | mix(c-2)
            st_tr = []   # awaiting transpose: (yt, img, s0)
            st_mix = []  # awaiting mix: (y, img, s0)
            for i, (img, s0) in enumerate(items):
                yt = emit_conv(img, s0)
                st_tr.append((yt, img, s0))
                if len(st_tr) > 1:
                    yt1, im1, sp1 = st_tr.pop(0)
                    st_mix.append((emit_tr(yt1), im1, sp1))
                if len(st_mix) > 1:
                    y1, im1, sp1 = st_mix.pop(0)
                    emit_mix(y1, im1, sp1 // 128)
            yt1, im1, sp1 = st_tr.pop(0)
            st_mix.append((emit_tr(yt1), im1, sp1))
            for y1, im1, sp1 in st_mix:
                emit_mix(y1, im1, sp1 // 128)

    nc.compile()
    return nc


_NC_CACHE = None


def kernel(x, dictionary, lookup_indices, lookup_coefficients):
    global _NC_CACHE
    import ml_dtypes
    from concourse import bass_utils

    bf16 = ml_dtypes.bfloat16

    x = np.asarray(x, dtype=np.float32)
    dictionary = np.asarray(dictionary, dtype=np.float32)
    idx = np.asarray(lookup_indices).astype(np.int64)
    coef = np.asarray(lookup_coefficients, dtype=np.float32)

    # M^T[d, o] = sum_s coeff[o, s] * [idx[o, s] == d]
    mt = np.zeros((NDICT, COUT), np.float32)
    np.add.at(mt, (idx.reshape(-1),
                   np.repeat(np.arange(COUT), S)), coef.reshape(-1))

    # wt[c_in_block, (cblk, ti, tj, d)] = dictionary[d, cblk*128+c, ti, tj]
    wt = np.ascontiguousarray(
        dictionary.reshape(NDICT, 2, 128, 3, 3).transpose(2, 1, 3, 4, 0)
    ).reshape(128, 2 * 9 * NDICT)

    # x -> [core, cin_in_block, img, cblk, padded-flat] (the SBUF layout, so
    # the device DMA is a plain contiguous copy)
    xp = np.zeros((16, CIN, HP, STR), np.float32)
    xp[:, :, 1:57, 1:57] = x
    xp = xp.reshape(16, CIN, HP * STR)
    xp = np.pad(xp, ((0, 0), (0, 0), (0, FLAT_X - HP * STR)))
    xp = xp.reshape(N_CORES, IMGS_PER_CORE, 2, 128, FLAT_X)
    xp = np.ascontiguousarray(xp.transpose(0, 3, 1, 2, 4))

    if _NC_CACHE is None:
        _NC_CACHE = _build_program()
    nc = _NC_CACHE

    in_maps = [{"x": xp[i].astype(bf16), "w": wt.astype(bf16),
                "m": mt.astype(bf16)} for i in range(N_CORES)]
    try:
        res = bass_utils.run_bass_kernel_spmd(
            nc, in_maps, core_ids=list(range(N_CORES)), trace=TRACE)
    except ModuleNotFoundError:
        # no axon NTFF profile hook in this environment
        res = bass_utils.run_bass_kernel_spmd(
            nc, in_maps, core_ids=list(range(N_CORES)), trace=False)
    _LAST_RESULTS["res"] = res

    # [core, img, chunk, 128, cout] -> strip pads, (16, 512, 56, 56) fp32
    out = np.stack([r["out"] for r in res.results], axis=0)
    out = out.reshape(N_CORES, IMGS_PER_CORE, NCHUNK * 128, COUT)
    valid = (np.arange(H)[:, None] * STR + np.arange(W)[None, :]).ravel()
    out = out.astype(np.float32)[:, :, valid, :]
    return np.ascontiguousarray(
        out.transpose(0, 1, 3, 2)).reshape(16, COUT, H, W)
